# revision 43
# baseline (speedup 1.0000x reference)
"""Multi-head causal attention (B=2,S=2048,E=1024,H=16,D=64) on 8 NeuronCores.

Sharding: core c handles batch b=c//4 and head-group hg=c%4 (4 heads = 256
channels each).  Each core computes Q^T/K^T/V projections for its channel
slice, causal softmax attention for its 4 heads, and a partial output
projection through its slice of Wo.  Host sums the 4 partials per batch and
adds the bias.

Pipeline (pair-structured, transpose-free):
  - Q^T/K^T live as head-PAIR tiles [128, S]: head 2p in partitions 0:64,
    head 2p+1 in 64:128.  Scores for a pair are TWO row-tiled K=64 matmuls
    (tile_position (0,0)/(64,0)) running concurrently on the PE array into
    one 2-bank PSUM tile [128, 2*512].
  - One ACT exp instruction per pair-chunk covers both heads ([128,2,w]
    strided view), halving ScalarE instruction overhead.
  - AV for a pair is TWO col-tiled M=64 matmuls ((0,0)/(0,64)) into one
    PSUM bank (head A in partitions 0:64, head B in 64:128).
  - Softmax denominators: four M=1 ones-matmuls (col strips 0/32/64/96),
    one concurrent array pass per chunk-index, accumulated in one bank.
  - Normalize: DVE reciprocal of the denominator bank, then K=1 broadcast
    matmuls (row strips 32h) re-using the denominator bank, then one DVE
    mul per pair into ons.
Dense projection matmuls are interleaved as fillers so the PE never sees
a >3.4us idle window (HAM stays at 2.4 GHz).
"""

import sys

sys.path.insert(0, "/opt/trn_rl_repo")

import numpy as np

B, S, E, H, D = 2, 2048, 1024, 16, 64
N_CORES = 8
HPC = 4               # heads per core
CH = HPC * D          # 256 channels per core
SBK = 512             # seq block (moving free dim)
NSB = S // SBK        # 4
NE = E // 128         # 8 contraction chunks
NKC = S // 128        # 16 key chunks

_BUILT = {}
DEBUG = False


def _build():
    if "nc" in _BUILT:
        return _BUILT["nc"]

    from contextlib import ExitStack

    import concourse.bacc as bacc
    import concourse.tile as tile
    from concourse import mybir

    F32 = mybir.dt.float32
    BF16 = mybir.dt.bfloat16
    FP8 = mybir.dt.float8e4
    AF = mybir.ActivationFunctionType

    nc = bacc.Bacc("TRN2", target_bir_lowering=False, debug=False,
                   num_devices=N_CORES)
    # host pre-packs [E, C] as [128, NE*C] (row 128c+p -> col c*C+j)
    xt = nc.dram_tensor("xt", [128, NE * S], BF16, kind="ExternalInput").ap()
    wq = nc.dram_tensor("wq", [128, NE * CH], BF16, kind="ExternalInput").ap()
    wk = nc.dram_tensor("wk", [128, NE * CH], BF16, kind="ExternalInput").ap()
    wv = nc.dram_tensor("wv", [128, NE * CH], BF16, kind="ExternalInput").ap()
    wo = nc.dram_tensor("wo", [CH, E], BF16, kind="ExternalInput").ap()
    tri = nc.dram_tensor("tri", [128, 128], BF16, kind="ExternalInput").ap()
    pout = nc.dram_tensor("pout", [S, E], F32, kind="ExternalOutput").ap()
    dbg = {}
    if DEBUG:
        for nm, shp, dt in (("dqt0", [128, S], BF16), ("dkt0", [128, S], BF16),
                            ("dvt0", [128, CH], BF16),
                            ("dpt", [128, 1024], BF16),
                            ("dav", [128, 512], F32), ("dden", [128, 512], F32),
                            ("dbc", [128, 512], F32), ("don0", [128, S], BF16)):
            dbg[nm] = nc.dram_tensor(nm, shp, dt, kind="ExternalOutput").ap()

    with tile.TileContext(nc) as tc, ExitStack() as ctx:
        trip = ctx.enter_context(tc.tile_pool(name="trip", bufs=1))
        xtp = ctx.enter_context(tc.tile_pool(name="xtp", bufs=1))
        wp = ctx.enter_context(tc.tile_pool(name="wp", bufs=1))
        wop = ctx.enter_context(tc.tile_pool(name="wop", bufs=2))
        qkp = ctx.enter_context(tc.tile_pool(name="qkp", bufs=4))
        vp = ctx.enter_context(tc.tile_pool(name="vp", bufs=NKC))
        onp = ctx.enter_context(tc.tile_pool(name="onp", bufs=2))
        ptp = ctx.enter_context(tc.tile_pool(name="ptp", bufs=6))
        recp = ctx.enter_context(tc.tile_pool(name="recp", bufs=2))
        oop = ctx.enter_context(tc.tile_pool(name="oop", bufs=6))
        bcp = ctx.enter_context(tc.tile_pool(name="bcp", bufs=4))
        rc1p = ctx.enter_context(tc.tile_pool(name="rc1p", bufs=4))
        # PSUM: 2*2 (ss) + 2 (av) + 1 (den/bcast) + 1 (fillers) = 8 banks
        ssp = ctx.enter_context(tc.tile_pool(name="ssp", bufs=2, space="PSUM"))
        avp = ctx.enter_context(tc.tile_pool(name="avp", bufs=2, space="PSUM"))
        denp = ctx.enter_context(tc.tile_pool(name="denp", bufs=1,
                                              space="PSUM"))
        ppp = ctx.enter_context(tc.tile_pool(name="ppp", bufs=1, space="PSUM"))

        # --- constants + loads; x^T columns 0:512 first (critical path) ---
        tri_sb = trip.tile([128, 128], BF16, tag="tri")
        nc.gpsimd.dma_start(tri_sb[:], tri[:, :])
        ones_bf = trip.tile([128, 1], BF16, tag="ones_bf")
        nc.vector.memset(ones_bf[:], 1.0)
        ones_f32 = trip.tile([128, 64], F32, tag="ones_f32")
        nc.vector.memset(ones_f32[:], 1.0)
        wrm = trip.tile([128, 128], BF16, tag="wrm")
        nc.vector.memset(wrm[:], 0.125)
        xtt = xtp.tile([128, NE * S], BF16, tag="xt")

        def xsl(e, c0, w):
            # packed col order: sb-major [sb, e, s] (s within one 512-block)
            sb, s0 = c0 // SBK, c0 % SBK
            base = sb * NE * SBK + e * SBK + s0
            return xtt[:, base:base + w]
        wqt = wp.tile([128, NE * CH], BF16, tag="wq")
        wkt = wp.tile([128, NE * CH], BF16, tag="wk")
        wvt = wp.tile([128, NE * CH], BF16, tag="wv")
        wqs = [wqt[:, e * CH:(e + 1) * CH] for e in range(NE)]
        wks = [wkt[:, e * CH:(e + 1) * CH] for e in range(NE)]
        wvs = [wvt[:, e * CH:(e + 1) * CH] for e in range(NE)]
        HW_ = NE * CH // 2
        nc.scalar.dma_start(wqt[:, 0:HW_], wq[:, 0:HW_])
        nc.scalar.dma_start(wqt[:, HW_:], wq[:, HW_:])
        nc.gpsimd.dma_start(wkt[:, 0:HW_], wk[:, 0:HW_])
        nc.gpsimd.dma_start(wkt[:, HW_:], wk[:, HW_:])
        BLK = NE * SBK
        QB_ = BLK // 4
        for i in range(4):  # sb0 in quarters (first consumers)
            nc.sync.dma_start(xtt[:, i * QB_:(i + 1) * QB_],
                              xt[:, i * QB_:(i + 1) * QB_])
        nc.gpsimd.dma_start(wvt[:], wv[:, :])
        for sb in range(1, NSB):
            nc.sync.dma_start(xtt[:, sb * BLK:(sb + 1) * BLK],
                              xt[:, sb * BLK:(sb + 1) * BLK])
        wos = []
        for p in range(2):
            t = wop.tile([128, E], BF16, tag="wo")
            nc.gpsimd.dma_start(t[:], wo[p * 128:(p + 1) * 128, :])
            wos.append(t)

        # q/k head-pair tiles [128, S]; vts [128, CH] per key chunk
        qkt = {"q": [qkp.tile([128, S], BF16, tag="qk", name=f"qt{p}")
                     for p in range(2)],
               "k": [qkp.tile([128, S], BF16, tag="qk", name=f"kt{p}")
                     for p in range(2)]}
        vts = [vp.tile([128, CH], BF16, tag="v", name=f"v{i}")
               for i in range(NKC)]
        ons = [onp.tile([128, S], BF16, tag="on", name=f"on{p}")
               for p in range(2)]

        # ---- dense-matmul group emitters (PE filler work) ----
        def qk_group(name, p, sb, pool=None, tag="pp"):
            ps = (pool or ppp).tile([128, SBK], F32, tag=tag,
                                    name=f"ps_{name}{p}{sb}")
            wts = wqs if name == "q" else wks
            for e in range(NE):
                nc.tensor.matmul(
                    ps[:], lhsT=wts[e][:, p * 128:(p + 1) * 128],
                    rhs=xsl(e, sb * SBK, SBK),
                    start=(e == 0), stop=(e == NE - 1))
            nc.vector.tensor_copy(qkt[name][p][:, sb * SBK:(sb + 1) * SBK],
                                  ps[:])

        def v_group(sc, pool=None):
            ps = (pool or ppp).tile([128, CH], F32, tag="pp", name=f"ps_v{sc}")
            for e in range(NE):
                nc.tensor.matmul(ps[:], lhsT=xsl(e, sc * 128, 128),
                                 rhs=wvs[e], start=(e == 0),
                                 stop=(e == NE - 1))
            nc.vector.tensor_copy(vts[sc][:], ps[:])

        def wo_group(sc, eb, pool=None, tag="pp", tail=False):
            ps = (pool or ppp).tile([128, SBK], F32, tag=tag,
                                    name=f"ps_o{sc}{eb}")
            for p in range(2):
                nc.tensor.matmul(ps[:],
                                 lhsT=ons[p][:, sc * 128:(sc + 1) * 128],
                                 rhs=wos[p][:, eb * SBK:(eb + 1) * SBK],
                                 start=(p == 0), stop=(p == 1))
            oo = oop.tile([128, SBK], F32, tag="oo", name=f"oo{sc}{eb}")
            if tail and eb % 2 == 0:
                nc.scalar.copy(oo[:], ps[:])
            else:
                nc.vector.tensor_copy(oo[:], ps[:])
            (nc.gpsimd if (tail and eb % 2 == 0) else nc.sync).dma_start(
                pout[sc * 128:(sc + 1) * 128, eb * SBK:(eb + 1) * SBK],
                oo[:])

        from collections import deque
        fillers = deque()  # entries: (tag, est_pe_cost_ns, fn)
        QK_COST, V_COST, WO_COST = 1750, 900, 500

        def pop_fillers(budget_ns):
            while fillers and budget_ns > 0:
                tag, cost, fn = fillers.popleft()
                fn()
                budget_ns -= cost

        def ensure(tag):
            # force-run a queued filler the upcoming consumer depends on
            for i, (t, cost, fn) in enumerate(fillers):
                if t == tag:
                    del fillers[i]
                    fn()
                    return

        # dep-free warm-up: keeps PE at 2.4 GHz while input DMAs land
        warm = denp.tile([128, 512], F32, tag="den", name="warm")
        for _ in range(45):
            nc.tensor.matmul(warm[:, 0:128], lhsT=wrm[:], rhs=wrm[:],
                             start=True, stop=True)
        # prologue: only what (qb0, kc0) needs; everything else is filler
        qk_group("q", 0, 0, pool=avp, tag="av")
        qk_group("k", 0, 0, pool=avp, tag="av")
        qk_group("q", 1, 0, pool=avp, tag="av")
        qk_group("k", 1, 0, pool=avp, tag="av")
        for sc in range(4):
            fillers.append((("v", sc), V_COST, lambda sc=sc: v_group(sc)))
        for sb in range(1, NSB):
            for nm in ("q", "k"):
                for p in range(2):
                    fillers.append(
                        (("qk", nm, p, sb), QK_COST,
                         lambda nm=nm, p=p, sb=sb: qk_group(nm, p, sb)))

        SCALE = float(D) ** -0.5
        pending_norm = [None]  # deferred normalize emitter for previous qb

        # ---- attention ----
        for qb in range(NSB):
            nk = 4 * (qb + 1)
            if qb > 0:  # guarantee this qb's q/k blocks exist before scores
                for nm in ("q", "k"):
                    for p in range(2):
                        ensure(("qk", nm, p, qb))
            av = [avp.tile([128, SBK], F32, tag="av", name=f"av{qb}{p}")
                  for p in range(2)]
            den = denp.tile([128, 512], F32, tag="den", name=f"den{qb}")
            pend = None  # previous chunk payload: (kc, j0, [pt0, pt1])

            def flush(payload, last=False, av=av, den=den):
                kc, j0, pts = payload
                for p in range(2):
                    for hh in range(2):
                        nc.tensor.matmul(
                            av[p][hh * 64:hh * 64 + 64, j0:SBK],
                            lhsT=vts[kc][:, p * 128 + hh * 64:
                                         p * 128 + hh * 64 + 64],
                            rhs=pts[p][:, hh * SBK + j0:(hh + 1) * SBK],
                            start=False,
                            stop=(last and hh == 1),
                            skip_group_check=True)
                for h in range(HPC):
                    p, hh = h // 2, h % 2
                    nc.tensor.matmul(
                        den[32 * h:32 * h + 1, j0:SBK],
                        lhsT=ones_bf[:, 0:1],
                        rhs=pts[p][:, hh * SBK + j0:(hh + 1) * SBK],
                        start=False,
                        stop=(last and h == HPC - 1),
                        skip_group_check=True,
                        tile_position=(0, 32 * h))

            for kc in range(nk):
                k0 = kc * 128
                j0 = max(0, k0 - qb * SBK)
                pts = []
                for p in range(2):
                    ss = ssp.tile([128, 2 * SBK], F32, tag="ss",
                                  name=f"ss{qb}{kc}{p}")
                    qt, kt = qkt["q"][p], qkt["k"][p]
                    for hh in range(2):
                        nc.tensor.matmul(
                            ss[:, hh * SBK + j0:(hh + 1) * SBK],
                            lhsT=kt[hh * 64:hh * 64 + 64, k0:k0 + 128],
                            rhs=qt[hh * 64:hh * 64 + 64,
                                   qb * SBK + j0:(qb + 1) * SBK],
                            start=True, stop=True)
                    pt = ptp.tile([128, 2 * SBK], BF16, tag="pt",
                                  name=f"pt{qb}{kc}{p}")
                    ss3 = ss[:].rearrange("a (c q) -> a c q", c=2)[:, :, j0:SBK]
                    pt3 = pt[:].rearrange("a (c q) -> a c q", c=2)[:, :, j0:SBK]
                    nc.scalar.activation(pt3, ss3, AF.Exp, scale=SCALE)
                    if k0 >= qb * SBK:  # diag chunk: mask 128-wide band
                        band_pt = pt[:].rearrange(
                            "a (c q) -> a c q", c=2)[:, :, j0:j0 + 128]
                        tri_bc = tri_sb[:].unsqueeze(1).broadcast_to(
                            [128, 2, 128])
                        nc.vector.tensor_mul(band_pt, band_pt, tri_bc)
                    if DEBUG and qb == 0 and kc == 0 and p == 0:
                        nc.sync.dma_start(dbg["dpt"][:, :], pt[:])
                    pts.append(pt)
                if kc == 0:
                    if pending_norm[0] is not None:
                        pending_norm[0]()
                        pending_norm[0] = None
                    nc.vector.memset(av[0][:], 0.0)
                    nc.vector.memset(av[1][:], 0.0)
                    nc.vector.memset(den[:], 0.0)
                if pend is not None:
                    ensure(("v", pend[0]))
                    flush(pend)
                pend = (kc, j0, pts)
                pop_fillers(1900 if qb == 0 else 1150)
            ensure(("v", pend[0]))
            flush(pend, last=True)
            if DEBUG and qb == 0:
                dav_sb = oop.tile([128, SBK], F32, tag="oo", name="dav_sb")
                nc.vector.tensor_copy(dav_sb[:], av[0][:])
                nc.sync.dma_start(dbg["dav"][:, :], dav_sb[:])
                dden_sb = oop.tile([128, SBK], F32, tag="oo", name="dden_sb")
                nc.vector.tensor_copy(dden_sb[:], den[:])
                nc.sync.dma_start(dbg["dden"][:, :], dden_sb[:])
            pop_fillers(700)  # cover the reciprocal latency

            def make_norm(qb=qb, av=av, den=den, last=(qb == NSB - 1)):
                def norm():
                    # recip -> PE bcast matmuls (den bank re-used) -> mul
                    rec = recp.tile([128, 512], F32, tag="rec",
                                    name=f"rec{qb}")
                    nc.vector.reciprocal_approx_fast(rec[:], den[:])
                    for p in range(2):
                        for hh in range(2):
                            h = 2 * p + hh
                            nc.tensor.matmul(
                                den[hh * 64:hh * 64 + 64, :],
                                lhsT=ones_f32[32 * h:32 * h + 1, 0:64],
                                rhs=rec[32 * h:32 * h + 1, :],
                                start=True, stop=True,
                                skip_group_check=True,
                                tile_position=(32 * h, 64 * hh))
                        bc = bcp.tile([128, 512], F32, tag="bc",
                                      name=f"bc{qb}{p}")
                        nc.vector.tensor_copy(bc[:], den[:])
                        if DEBUG and qb == 0 and p == 0:
                            nc.sync.dma_start(dbg["dbc"][:, :], bc[:])
                        nc.vector.tensor_mul(
                            ons[p][:, qb * SBK:(qb + 1) * SBK],
                            av[p][:], bc[:])
                    # wo fillers for this qb are dep-safe only from here on
                    for sc in range(4 * qb, 4 * (qb + 1)):
                        for eb in range(2):
                            fillers.append(
                                (("wo", sc, eb), WO_COST,
                                 lambda sc=sc, eb=eb: wo_group(sc, eb)))
                return norm

            pending_norm[0] = make_norm()
            if qb + 1 < NSB:
                for sc in reversed(range(4 * (qb + 1), 4 * (qb + 2))):
                    fillers.appendleft(
                        (("v", sc), V_COST, lambda sc=sc: v_group(sc)))

        tail_pools = [(ppp, "pp"), (ssp, "ss"), (avp, "av"), (denp, "den"),
                      (ssp, "ss"), (avp, "av")]
        tail_i = [0]

        def tail_wo(sc, eb):
            pool, tg = tail_pools[tail_i[0] % len(tail_pools)]
            tail_i[0] += 1
            wo_group(sc, eb, pool=pool, tag=tg, tail=True)

        pending_norm[0]()
        if DEBUG:
            nc.gpsimd.dma_start(dbg["dqt0"][:, :], qkt["q"][0][:])
            nc.gpsimd.dma_start(dbg["dkt0"][:, :], qkt["k"][0][:])
            nc.sync.dma_start(dbg["dvt0"][:, :], vts[0][:])
            nc.sync.dma_start(dbg["don0"][:, :], ons[0][:])
        while fillers:
            tag, cost, fn = fillers.popleft()
            if tag[0] == "wo":
                tail_wo(tag[1], tag[2])
            else:
                fn()

    nc.compile()
    _BUILT["nc"] = nc
    return nc


def _install_ntff_shim():
    """Provide antenv.axon_hooks (missing in this image) so trace=True works."""
    import types
    try:
        from antenv.axon_hooks import get_axon_ntff_profile_hook  # noqa: F401
        return
    except ImportError:
        pass
    import antenv
    from trn_agent_boot.trn_boot import _ntff_profile_via_ctypes
    hook = _ntff_profile_via_ctypes("/opt/axon/libaxon_pjrt.so")
    mod = types.ModuleType("antenv.axon_hooks")
    mod._hook = hook
    mod.get_axon_ntff_profile_hook = lambda: mod._hook
    mod.set_axon_ntff_profile_hook = lambda h: setattr(mod, "_hook", h)
    sys.modules["antenv.axon_hooks"] = mod
    antenv.axon_hooks = mod


def kernel(x, Wq, Wk, Wv, Wo, bo, _trace=False):
    from concourse.bass_utils import run_bass_kernel_spmd

    nc = _build()

    x = np.asarray(x, dtype=np.float32)
    Wq = np.asarray(Wq, dtype=np.float32)
    Wk = np.asarray(Wk, dtype=np.float32)
    Wv = np.asarray(Wv, dtype=np.float32)
    Wo = np.asarray(Wo, dtype=np.float32)
    bo = np.asarray(bo, dtype=np.float32)

    import ml_dtypes
    bf = ml_dtypes.bfloat16
    tri = np.triu(np.ones((128, 128), dtype=np.float32)).astype(bf)

    def pack8(a):
        # [1024, C] -> [128, 8*C]: row 128c+p -> col c*C+j
        C = a.shape[1]
        return np.ascontiguousarray(
            a.reshape(8, 128, C).transpose(1, 0, 2).reshape(128, 8 * C))

    def packx(a):  # x^T [E,S] -> [128, (sb, e, s)] sb-major
        return np.ascontiguousarray(
            a.reshape(8, 128, 4, 512).transpose(1, 2, 0, 3).reshape(
                128, 8 * 2048))

    xt_b = [packx(np.ascontiguousarray(x[b].T)).astype(bf) for b in range(B)]
    in_maps = []
    for c in range(N_CORES):
        b, hg = c // HPC, c % HPC
        sl = slice(hg * CH, (hg + 1) * CH)
        in_maps.append({
            "xt": xt_b[b],
            "wq": pack8(np.ascontiguousarray(Wq[:, sl])).astype(bf),
            "wk": pack8(np.ascontiguousarray(Wk[:, sl])).astype(bf),
            "wv": pack8(np.ascontiguousarray(Wv[:, sl])).astype(bf),
            "wo": np.ascontiguousarray(Wo[sl, :]).astype(bf),
            "tri": tri,
        })

    kwargs = {}
    if _trace:
        _install_ntff_shim()
        kwargs = dict(trace=True, trace_cores=[0])
    res = run_bass_kernel_spmd(nc, in_maps, core_ids=list(range(N_CORES)),
                               **kwargs)

    out = np.zeros((B, S, E), dtype=np.float32)
    for c in range(N_CORES):
        out[c // HPC] += res.results[c]["pout"]
    out += bo
    if _trace:
        return out, res
    return out


# revision 44
# speedup vs baseline: 1.0157x; 1.0157x over previous
"""Multi-head causal attention (B=2,S=2048,E=1024,H=16,D=64) on 8 NeuronCores.

Sharding: core c handles batch b=c//4 and head-group hg=c%4 (4 heads = 256
channels each).  Each core computes Q^T/K^T/V projections for its channel
slice, causal softmax attention for its 4 heads, and a partial output
projection through its slice of Wo.  Host sums the 4 partials per batch and
adds the bias.

Pipeline (pair-structured, transpose-free):
  - Q^T/K^T live as head-PAIR tiles [128, S]: head 2p in partitions 0:64,
    head 2p+1 in 64:128.  Scores for a pair are TWO row-tiled K=64 matmuls
    (tile_position (0,0)/(64,0)) running concurrently on the PE array into
    one 2-bank PSUM tile [128, 2*512].
  - One ACT exp instruction per pair-chunk covers both heads ([128,2,w]
    strided view), halving ScalarE instruction overhead.
  - AV for a pair is TWO col-tiled M=64 matmuls ((0,0)/(0,64)) into one
    PSUM bank (head A in partitions 0:64, head B in 64:128).
  - Softmax denominators: four M=1 ones-matmuls (col strips 0/32/64/96),
    one concurrent array pass per chunk-index, accumulated in one bank.
  - Normalize: DVE reciprocal of the denominator bank, then K=1 broadcast
    matmuls (row strips 32h) re-using the denominator bank, then one DVE
    mul per pair into ons.
Dense projection matmuls are interleaved as fillers so the PE never sees
a >3.4us idle window (HAM stays at 2.4 GHz).
"""

import sys

sys.path.insert(0, "/opt/trn_rl_repo")

import numpy as np

B, S, E, H, D = 2, 2048, 1024, 16, 64
N_CORES = 8
HPC = 4               # heads per core
CH = HPC * D          # 256 channels per core
SBK = 512             # seq block (moving free dim)
NSB = S // SBK        # 4
NE = E // 128         # 8 contraction chunks
NKC = S // 128        # 16 key chunks

_BUILT = {}
DEBUG = False


def _build():
    if "nc" in _BUILT:
        return _BUILT["nc"]

    from contextlib import ExitStack

    import concourse.bacc as bacc
    import concourse.tile as tile
    from concourse import mybir

    F32 = mybir.dt.float32
    BF16 = mybir.dt.bfloat16
    FP8 = mybir.dt.float8e4
    AF = mybir.ActivationFunctionType

    nc = bacc.Bacc("TRN2", target_bir_lowering=False, debug=False,
                   num_devices=N_CORES)
    # host pre-packs [E, C] as [128, NE*C] (row 128c+p -> col c*C+j)
    xt = nc.dram_tensor("xt", [128, NE * S], BF16, kind="ExternalInput").ap()
    wq = nc.dram_tensor("wq", [128, NE * CH], BF16, kind="ExternalInput").ap()
    wk = nc.dram_tensor("wk", [128, NE * CH], BF16, kind="ExternalInput").ap()
    wv = nc.dram_tensor("wv", [128, NE * CH], BF16, kind="ExternalInput").ap()
    wo = nc.dram_tensor("wo", [CH, E], BF16, kind="ExternalInput").ap()
    tri = nc.dram_tensor("tri", [128, 128], BF16, kind="ExternalInput").ap()
    pout = nc.dram_tensor("pout", [S, E], F32, kind="ExternalOutput").ap()
    dbg = {}
    if DEBUG:
        for nm, shp, dt in (("dqt0", [128, S], BF16), ("dkt0", [128, S], BF16),
                            ("dvt0", [128, CH], BF16),
                            ("dpt", [128, 1024], BF16),
                            ("dav", [128, 512], F32), ("dden", [128, 512], F32),
                            ("dbc", [128, 512], F32), ("don0", [128, S], BF16)):
            dbg[nm] = nc.dram_tensor(nm, shp, dt, kind="ExternalOutput").ap()

    with tile.TileContext(nc) as tc, ExitStack() as ctx:
        trip = ctx.enter_context(tc.tile_pool(name="trip", bufs=1))
        xtp = ctx.enter_context(tc.tile_pool(name="xtp", bufs=1))
        wp = ctx.enter_context(tc.tile_pool(name="wp", bufs=1))
        wop = ctx.enter_context(tc.tile_pool(name="wop", bufs=2))
        qkp = ctx.enter_context(tc.tile_pool(name="qkp", bufs=4))
        vp = ctx.enter_context(tc.tile_pool(name="vp", bufs=NKC))
        onp = ctx.enter_context(tc.tile_pool(name="onp", bufs=2))
        ptp = ctx.enter_context(tc.tile_pool(name="ptp", bufs=6))
        recp = ctx.enter_context(tc.tile_pool(name="recp", bufs=2))
        oop = ctx.enter_context(tc.tile_pool(name="oop", bufs=6))
        bcp = ctx.enter_context(tc.tile_pool(name="bcp", bufs=4))
        rc1p = ctx.enter_context(tc.tile_pool(name="rc1p", bufs=4))
        # PSUM: 2*2 (ss) + 2 (av) + 1 (den/bcast) + 1 (fillers) = 8 banks
        ssp = ctx.enter_context(tc.tile_pool(name="ssp", bufs=2, space="PSUM"))
        avp = ctx.enter_context(tc.tile_pool(name="avp", bufs=2, space="PSUM"))
        denp = ctx.enter_context(tc.tile_pool(name="denp", bufs=1,
                                              space="PSUM"))
        ppp = ctx.enter_context(tc.tile_pool(name="ppp", bufs=1, space="PSUM"))

        # --- constants + loads; x^T columns 0:512 first (critical path) ---
        tri_sb = trip.tile([128, 128], BF16, tag="tri")
        nc.gpsimd.dma_start(tri_sb[:], tri[:, :])
        ones_bf = trip.tile([128, 1], BF16, tag="ones_bf")
        nc.vector.memset(ones_bf[:], 1.0)
        ones_f32 = trip.tile([128, 64], F32, tag="ones_f32")
        nc.vector.memset(ones_f32[:], 1.0)
        wrm = trip.tile([128, 128], BF16, tag="wrm")
        nc.vector.memset(wrm[:], 0.125)
        xtt = xtp.tile([128, NE * S], BF16, tag="xt")

        def xsl(e, c0, w):
            # packed col order: sb-major [sb, e, s] (s within one 512-block)
            sb, s0 = c0 // SBK, c0 % SBK
            base = sb * NE * SBK + e * SBK + s0
            return xtt[:, base:base + w]
        wqt = wp.tile([128, NE * CH], BF16, tag="wq")
        wkt = wp.tile([128, NE * CH], BF16, tag="wk")
        wvt = wp.tile([128, NE * CH], BF16, tag="wv")
        wqs = [wqt[:, e * CH:(e + 1) * CH] for e in range(NE)]
        wks = [wkt[:, e * CH:(e + 1) * CH] for e in range(NE)]
        wvs = [wvt[:, e * CH:(e + 1) * CH] for e in range(NE)]
        HW_ = NE * CH // 2
        nc.scalar.dma_start(wqt[:, 0:HW_], wq[:, 0:HW_])
        nc.scalar.dma_start(wqt[:, HW_:], wq[:, HW_:])
        nc.gpsimd.dma_start(wkt[:, 0:HW_], wk[:, 0:HW_])
        nc.gpsimd.dma_start(wkt[:, HW_:], wk[:, HW_:])
        BLK = NE * SBK
        QB_ = BLK // 4
        for i in range(4):  # sb0 in quarters (first consumers)
            nc.sync.dma_start(xtt[:, i * QB_:(i + 1) * QB_],
                              xt[:, i * QB_:(i + 1) * QB_])
        nc.gpsimd.dma_start(wvt[:], wv[:, :])
        for sb in range(1, NSB):
            nc.sync.dma_start(xtt[:, sb * BLK:(sb + 1) * BLK],
                              xt[:, sb * BLK:(sb + 1) * BLK])
        wos = []
        for p in range(2):
            t = wop.tile([128, E], BF16, tag="wo")
            nc.gpsimd.dma_start(t[:], wo[p * 128:(p + 1) * 128, :])
            wos.append(t)

        # q/k head-pair tiles [128, S]; vts [128, CH] per key chunk
        qkt = {"q": [qkp.tile([128, S], BF16, tag="qk", name=f"qt{p}")
                     for p in range(2)],
               "k": [qkp.tile([128, S], BF16, tag="qk", name=f"kt{p}")
                     for p in range(2)]}
        vts = [vp.tile([128, CH], BF16, tag="v", name=f"v{i}")
               for i in range(NKC)]
        ons = [onp.tile([128, S], BF16, tag="on", name=f"on{p}")
               for p in range(2)]

        # ---- dense-matmul group emitters (PE filler work) ----
        def qk_group(name, p, sb, pool=None, tag="pp"):
            ps = (pool or ppp).tile([128, SBK], F32, tag=tag,
                                    name=f"ps_{name}{p}{sb}")
            wts = wqs if name == "q" else wks
            for e in range(NE):
                nc.tensor.matmul(
                    ps[:], lhsT=wts[e][:, p * 128:(p + 1) * 128],
                    rhs=xsl(e, sb * SBK, SBK),
                    start=(e == 0), stop=(e == NE - 1))
            nc.vector.tensor_copy(qkt[name][p][:, sb * SBK:(sb + 1) * SBK],
                                  ps[:])

        def v_group(sc, pool=None):
            ps = (pool or ppp).tile([128, CH], F32, tag="pp", name=f"ps_v{sc}")
            for e in range(NE):
                nc.tensor.matmul(ps[:], lhsT=xsl(e, sc * 128, 128),
                                 rhs=wvs[e], start=(e == 0),
                                 stop=(e == NE - 1))
            nc.vector.tensor_copy(vts[sc][:], ps[:])

        def wo_group(sc, eb, pool=None, tag="pp", tail=False):
            ps = (pool or ppp).tile([128, SBK], F32, tag=tag,
                                    name=f"ps_o{sc}{eb}")
            for p in range(2):
                nc.tensor.matmul(ps[:],
                                 lhsT=ons[p][:, sc * 128:(sc + 1) * 128],
                                 rhs=wos[p][:, eb * SBK:(eb + 1) * SBK],
                                 start=(p == 0), stop=(p == 1))
            oo = oop.tile([128, SBK], F32, tag="oo", name=f"oo{sc}{eb}")
            if tail and eb % 2 == 0:
                nc.scalar.copy(oo[:], ps[:])
            else:
                nc.vector.tensor_copy(oo[:], ps[:])
            (nc.gpsimd if (tail and eb % 2 == 0) else nc.sync).dma_start(
                pout[sc * 128:(sc + 1) * 128, eb * SBK:(eb + 1) * SBK],
                oo[:])

        from collections import deque
        fillers = deque()  # entries: (tag, est_pe_cost_ns, fn)
        QK_COST, V_COST, WO_COST = 1750, 900, 500

        def pop_fillers(budget_ns):
            while fillers and budget_ns > 0:
                tag, cost, fn = fillers.popleft()
                fn()
                budget_ns -= cost

        def ensure(tag):
            # force-run a queued filler the upcoming consumer depends on
            for i, (t, cost, fn) in enumerate(fillers):
                if t == tag:
                    del fillers[i]
                    fn()
                    return

        # dep-free warm-up: keeps PE at 2.4 GHz while input DMAs land
        warm = denp.tile([128, 512], F32, tag="den", name="warm")
        for _ in range(45):
            nc.tensor.matmul(warm[:, 0:128], lhsT=wrm[:], rhs=wrm[:],
                             start=True, stop=True)
        # prologue: only what (qb0, kc0, pair0) needs; pair1 is ensured later
        qk_group("q", 0, 0, pool=avp, tag="av")
        qk_group("k", 0, 0, pool=avp, tag="av")
        for nm in ("q", "k"):
            fillers.append(
                (("qk", nm, 1, 0), QK_COST,
                 lambda nm=nm: qk_group(nm, 1, 0, pool=avp, tag="av")))
        for sc in range(4):
            fillers.append((("v", sc), V_COST, lambda sc=sc: v_group(sc)))
        for sb in range(1, NSB):
            for nm in ("q", "k"):
                for p in range(2):
                    fillers.append(
                        (("qk", nm, p, sb), QK_COST,
                         lambda nm=nm, p=p, sb=sb: qk_group(nm, p, sb)))

        SCALE = float(D) ** -0.5
        pending_norm = [None]  # deferred normalize emitter for previous qb

        # ---- attention ----
        for qb in range(NSB):
            nk = 4 * (qb + 1)
            if qb > 0:  # guarantee this qb's q/k blocks exist before scores
                for nm in ("q", "k"):
                    for p in range(2):
                        ensure(("qk", nm, p, qb))
            av = [avp.tile([128, SBK], F32, tag="av", name=f"av{qb}{p}")
                  for p in range(2)]
            den = denp.tile([128, 512], F32, tag="den", name=f"den{qb}")
            pend = None  # previous chunk payload: (kc, j0, [pt0, pt1])

            def flush(payload, last=False, av=av, den=den):
                kc, j0, pts = payload
                for p in range(2):
                    for hh in range(2):
                        nc.tensor.matmul(
                            av[p][hh * 64:hh * 64 + 64, j0:SBK],
                            lhsT=vts[kc][:, p * 128 + hh * 64:
                                         p * 128 + hh * 64 + 64],
                            rhs=pts[p][:, hh * SBK + j0:(hh + 1) * SBK],
                            start=False,
                            stop=(last and hh == 1),
                            skip_group_check=True)
                for h in range(HPC):
                    p, hh = h // 2, h % 2
                    nc.tensor.matmul(
                        den[32 * h:32 * h + 1, j0:SBK],
                        lhsT=ones_bf[:, 0:1],
                        rhs=pts[p][:, hh * SBK + j0:(hh + 1) * SBK],
                        start=False,
                        stop=(last and h == HPC - 1),
                        skip_group_check=True,
                        tile_position=(0, 32 * h))

            for kc in range(nk):
                k0 = kc * 128
                j0 = max(0, k0 - qb * SBK)
                pts = []
                for p in range(2):
                    if qb == 0 and kc == 0 and p == 1:
                        ensure(("qk", "q", 1, 0))
                        ensure(("qk", "k", 1, 0))
                    ss = ssp.tile([128, 2 * SBK], F32, tag="ss",
                                  name=f"ss{qb}{kc}{p}")
                    qt, kt = qkt["q"][p], qkt["k"][p]
                    for hh in range(2):
                        nc.tensor.matmul(
                            ss[:, hh * SBK + j0:(hh + 1) * SBK],
                            lhsT=kt[hh * 64:hh * 64 + 64, k0:k0 + 128],
                            rhs=qt[hh * 64:hh * 64 + 64,
                                   qb * SBK + j0:(qb + 1) * SBK],
                            start=True, stop=True)
                    pt = ptp.tile([128, 2 * SBK], BF16, tag="pt",
                                  name=f"pt{qb}{kc}{p}")
                    ss3 = ss[:].rearrange("a (c q) -> a c q", c=2)[:, :, j0:SBK]
                    pt3 = pt[:].rearrange("a (c q) -> a c q", c=2)[:, :, j0:SBK]
                    nc.scalar.activation(pt3, ss3, AF.Exp, scale=SCALE)
                    if k0 >= qb * SBK:  # diag chunk: mask 128-wide band
                        band_pt = pt[:].rearrange(
                            "a (c q) -> a c q", c=2)[:, :, j0:j0 + 128]
                        tri_bc = tri_sb[:].unsqueeze(1).broadcast_to(
                            [128, 2, 128])
                        nc.vector.tensor_mul(band_pt, band_pt, tri_bc)
                    if DEBUG and qb == 0 and kc == 0 and p == 0:
                        nc.sync.dma_start(dbg["dpt"][:, :], pt[:])
                    pts.append(pt)
                if kc == 0:
                    if pending_norm[0] is not None:
                        pending_norm[0]()
                        pending_norm[0] = None
                    nc.vector.memset(av[0][:], 0.0)
                    nc.vector.memset(av[1][:], 0.0)
                    nc.vector.memset(den[:], 0.0)
                if pend is not None:
                    ensure(("v", pend[0]))
                    flush(pend)
                pend = (kc, j0, pts)
                pop_fillers(1900 if qb == 0 else 1150)
            ensure(("v", pend[0]))
            flush(pend, last=True)
            if DEBUG and qb == 0:
                dav_sb = oop.tile([128, SBK], F32, tag="oo", name="dav_sb")
                nc.vector.tensor_copy(dav_sb[:], av[0][:])
                nc.sync.dma_start(dbg["dav"][:, :], dav_sb[:])
                dden_sb = oop.tile([128, SBK], F32, tag="oo", name="dden_sb")
                nc.vector.tensor_copy(dden_sb[:], den[:])
                nc.sync.dma_start(dbg["dden"][:, :], dden_sb[:])
            pop_fillers(700)  # cover the reciprocal latency

            def make_norm(qb=qb, av=av, den=den, last=(qb == NSB - 1)):
                def norm():
                    # recip -> PE bcast matmuls (den bank re-used) -> mul
                    rec = recp.tile([128, 512], F32, tag="rec",
                                    name=f"rec{qb}")
                    nc.vector.reciprocal_approx_fast(rec[:], den[:])
                    for p in range(2):
                        for hh in range(2):
                            h = 2 * p + hh
                            nc.tensor.matmul(
                                den[hh * 64:hh * 64 + 64, :],
                                lhsT=ones_f32[32 * h:32 * h + 1, 0:64],
                                rhs=rec[32 * h:32 * h + 1, :],
                                start=True, stop=True,
                                skip_group_check=True,
                                tile_position=(32 * h, 64 * hh))
                        bc = bcp.tile([128, 512], F32, tag="bc",
                                      name=f"bc{qb}{p}")
                        nc.vector.tensor_copy(bc[:], den[:])
                        if DEBUG and qb == 0 and p == 0:
                            nc.sync.dma_start(dbg["dbc"][:, :], bc[:])
                        nc.vector.tensor_mul(
                            ons[p][:, qb * SBK:(qb + 1) * SBK],
                            av[p][:], bc[:])
                    # wo fillers for this qb are dep-safe only from here on
                    for sc in range(4 * qb, 4 * (qb + 1)):
                        for eb in range(2):
                            fillers.append(
                                (("wo", sc, eb), WO_COST,
                                 lambda sc=sc, eb=eb: wo_group(sc, eb)))
                return norm

            pending_norm[0] = make_norm()
            if qb + 1 < NSB:
                for sc in reversed(range(4 * (qb + 1), 4 * (qb + 2))):
                    fillers.appendleft(
                        (("v", sc), V_COST, lambda sc=sc: v_group(sc)))

        tail_pools = [(ppp, "pp"), (ssp, "ss"), (avp, "av"), (denp, "den"),
                      (ssp, "ss"), (avp, "av")]
        tail_i = [0]

        def tail_wo(sc, eb):
            pool, tg = tail_pools[tail_i[0] % len(tail_pools)]
            tail_i[0] += 1
            wo_group(sc, eb, pool=pool, tag=tg, tail=True)

        pending_norm[0]()
        if DEBUG:
            nc.gpsimd.dma_start(dbg["dqt0"][:, :], qkt["q"][0][:])
            nc.gpsimd.dma_start(dbg["dkt0"][:, :], qkt["k"][0][:])
            nc.sync.dma_start(dbg["dvt0"][:, :], vts[0][:])
            nc.sync.dma_start(dbg["don0"][:, :], ons[0][:])
        while fillers:
            tag, cost, fn = fillers.popleft()
            if tag[0] == "wo":
                tail_wo(tag[1], tag[2])
            else:
                fn()

    nc.compile()
    _BUILT["nc"] = nc
    return nc


def _install_ntff_shim():
    """Provide antenv.axon_hooks (missing in this image) so trace=True works."""
    import types
    try:
        from antenv.axon_hooks import get_axon_ntff_profile_hook  # noqa: F401
        return
    except ImportError:
        pass
    import antenv
    from trn_agent_boot.trn_boot import _ntff_profile_via_ctypes
    hook = _ntff_profile_via_ctypes("/opt/axon/libaxon_pjrt.so")
    mod = types.ModuleType("antenv.axon_hooks")
    mod._hook = hook
    mod.get_axon_ntff_profile_hook = lambda: mod._hook
    mod.set_axon_ntff_profile_hook = lambda h: setattr(mod, "_hook", h)
    sys.modules["antenv.axon_hooks"] = mod
    antenv.axon_hooks = mod


def kernel(x, Wq, Wk, Wv, Wo, bo, _trace=False):
    from concourse.bass_utils import run_bass_kernel_spmd

    nc = _build()

    x = np.asarray(x, dtype=np.float32)
    Wq = np.asarray(Wq, dtype=np.float32)
    Wk = np.asarray(Wk, dtype=np.float32)
    Wv = np.asarray(Wv, dtype=np.float32)
    Wo = np.asarray(Wo, dtype=np.float32)
    bo = np.asarray(bo, dtype=np.float32)

    import ml_dtypes
    bf = ml_dtypes.bfloat16
    tri = np.triu(np.ones((128, 128), dtype=np.float32)).astype(bf)

    def pack8(a):
        # [1024, C] -> [128, 8*C]: row 128c+p -> col c*C+j
        C = a.shape[1]
        return np.ascontiguousarray(
            a.reshape(8, 128, C).transpose(1, 0, 2).reshape(128, 8 * C))

    def packx(a):  # x^T [E,S] -> [128, (sb, e, s)] sb-major
        return np.ascontiguousarray(
            a.reshape(8, 128, 4, 512).transpose(1, 2, 0, 3).reshape(
                128, 8 * 2048))

    xt_b = [packx(np.ascontiguousarray(x[b].T)).astype(bf) for b in range(B)]
    in_maps = []
    for c in range(N_CORES):
        b, hg = c // HPC, c % HPC
        sl = slice(hg * CH, (hg + 1) * CH)
        in_maps.append({
            "xt": xt_b[b],
            "wq": pack8(np.ascontiguousarray(Wq[:, sl])).astype(bf),
            "wk": pack8(np.ascontiguousarray(Wk[:, sl])).astype(bf),
            "wv": pack8(np.ascontiguousarray(Wv[:, sl])).astype(bf),
            "wo": np.ascontiguousarray(Wo[sl, :]).astype(bf),
            "tri": tri,
        })

    kwargs = {}
    if _trace:
        _install_ntff_shim()
        kwargs = dict(trace=True, trace_cores=[0])
    res = run_bass_kernel_spmd(nc, in_maps, core_ids=list(range(N_CORES)),
                               **kwargs)

    out = np.zeros((B, S, E), dtype=np.float32)
    for c in range(N_CORES):
        out[c // HPC] += res.results[c]["pout"]
    out += bo
    if _trace:
        return out, res
    return out


# revision 45
# speedup vs baseline: 1.0649x; 1.0484x over previous
"""Multi-head causal attention (B=2,S=2048,E=1024,H=16,D=64) on 8 NeuronCores.

Sharding: core c handles batch b=c//4 and head-group hg=c%4 (4 heads = 256
channels each).  Each core computes Q^T/K^T/V projections for its channel
slice, causal softmax attention for its 4 heads, and a partial output
projection through its slice of Wo.  Host sums the 4 partials per batch and
adds the bias.

Pipeline (pair-structured, transpose-free):
  - Q^T/K^T live as head-PAIR tiles [128, S]: head 2p in partitions 0:64,
    head 2p+1 in 64:128.  Scores for a pair are TWO row-tiled K=64 matmuls
    (tile_position (0,0)/(64,0)) running concurrently on the PE array into
    one 2-bank PSUM tile [128, 2*512].
  - One ACT exp instruction per pair-chunk covers both heads ([128,2,w]
    strided view), halving ScalarE instruction overhead.
  - AV for a pair is TWO col-tiled M=64 matmuls ((0,0)/(0,64)) into one
    PSUM bank (head A in partitions 0:64, head B in 64:128).
  - Softmax denominators: four M=1 ones-matmuls (col strips 0/32/64/96),
    one concurrent array pass per chunk-index, accumulated in one bank.
  - Normalize: DVE reciprocal of the denominator bank, then K=1 broadcast
    matmuls (row strips 32h) re-using the denominator bank, then one DVE
    mul per pair into ons.
Dense projection matmuls are interleaved as fillers so the PE never sees
a >3.4us idle window (HAM stays at 2.4 GHz).
"""

import sys

sys.path.insert(0, "/opt/trn_rl_repo")

import numpy as np

B, S, E, H, D = 2, 2048, 1024, 16, 64
N_CORES = 8
HPC = 4               # heads per core
CH = HPC * D          # 256 channels per core
SBK = 512             # seq block (moving free dim)
NSB = S // SBK        # 4
NE = E // 128         # 8 contraction chunks
NKC = S // 128        # 16 key chunks

_BUILT = {}
DEBUG = False


def _build():
    if "nc" in _BUILT:
        return _BUILT["nc"]

    from contextlib import ExitStack

    import concourse.bacc as bacc
    import concourse.tile as tile
    from concourse import mybir

    F32 = mybir.dt.float32
    BF16 = mybir.dt.bfloat16
    FP8 = mybir.dt.float8e4
    AF = mybir.ActivationFunctionType

    nc = bacc.Bacc("TRN2", target_bir_lowering=False, debug=False,
                   num_devices=N_CORES)
    # host pre-packs [E, C] as [128, NE*C] (row 128c+p -> col c*C+j)
    xt = nc.dram_tensor("xt", [128, NE * S], BF16, kind="ExternalInput").ap()
    wq = nc.dram_tensor("wq", [128, NE * CH], BF16, kind="ExternalInput").ap()
    wk8 = nc.dram_tensor("wk8", [4 * 128, 2 * CH], FP8,
                         kind="ExternalInput").ap()
    xt8 = nc.dram_tensor("xt8", [4 * 128, 2 * S], FP8,
                         kind="ExternalInput").ap()
    wk = nc.dram_tensor("wk", [128, NE * CH], BF16, kind="ExternalInput").ap()
    wv = nc.dram_tensor("wv", [128, NE * CH], BF16, kind="ExternalInput").ap()
    wo = nc.dram_tensor("wo", [CH, E], BF16, kind="ExternalInput").ap()
    tri = nc.dram_tensor("tri", [128, 128], BF16, kind="ExternalInput").ap()
    pout = nc.dram_tensor("pout", [S, E], F32, kind="ExternalOutput").ap()
    dbg = {}
    if DEBUG:
        for nm, shp, dt in (("dqt0", [128, S], BF16), ("dkt0", [128, S], BF16),
                            ("dvt0", [128, CH], BF16),
                            ("dpt", [128, 1024], BF16),
                            ("dav", [128, 512], F32), ("dden", [128, 512], F32),
                            ("dbc", [128, 512], F32), ("don0", [128, S], BF16)):
            dbg[nm] = nc.dram_tensor(nm, shp, dt, kind="ExternalOutput").ap()

    with tile.TileContext(nc) as tc, ExitStack() as ctx:
        trip = ctx.enter_context(tc.tile_pool(name="trip", bufs=1))
        xtp = ctx.enter_context(tc.tile_pool(name="xtp", bufs=1))
        wp = ctx.enter_context(tc.tile_pool(name="wp", bufs=1))
        wop = ctx.enter_context(tc.tile_pool(name="wop", bufs=2))
        qkp = ctx.enter_context(tc.tile_pool(name="qkp", bufs=4))
        vp = ctx.enter_context(tc.tile_pool(name="vp", bufs=NKC))
        onp = ctx.enter_context(tc.tile_pool(name="onp", bufs=2))
        ptp = ctx.enter_context(tc.tile_pool(name="ptp", bufs=6))
        recp = ctx.enter_context(tc.tile_pool(name="recp", bufs=2))
        oop = ctx.enter_context(tc.tile_pool(name="oop", bufs=6))
        bcp = ctx.enter_context(tc.tile_pool(name="bcp", bufs=4))
        x8p = ctx.enter_context(tc.tile_pool(name="x8p", bufs=4))
        w8p = ctx.enter_context(tc.tile_pool(name="w8p", bufs=4))
        rc1p = ctx.enter_context(tc.tile_pool(name="rc1p", bufs=4))
        # PSUM: 2*2 (ss) + 2 (av) + 1 (den/bcast) + 1 (fillers) = 8 banks
        ssp = ctx.enter_context(tc.tile_pool(name="ssp", bufs=2, space="PSUM"))
        avp = ctx.enter_context(tc.tile_pool(name="avp", bufs=2, space="PSUM"))
        denp = ctx.enter_context(tc.tile_pool(name="denp", bufs=1,
                                              space="PSUM"))
        ppp = ctx.enter_context(tc.tile_pool(name="ppp", bufs=1, space="PSUM"))

        # --- constants + loads; x^T columns 0:512 first (critical path) ---
        tri_sb = trip.tile([128, 128], BF16, tag="tri")
        nc.gpsimd.dma_start(tri_sb[:], tri[:, :])
        ones_bf = trip.tile([128, 1], BF16, tag="ones_bf")
        nc.vector.memset(ones_bf[:], 1.0)
        ones_f32 = trip.tile([128, 64], F32, tag="ones_f32")
        nc.vector.memset(ones_f32[:], 1.0)
        wrm = trip.tile([128, 128], BF16, tag="wrm")
        nc.vector.memset(wrm[:], 0.125)
        xtt = xtp.tile([128, NE * S], BF16, tag="xt")

        def xsl(e, c0, w):
            # packed col order: sb-major [sb, e, s] (s within one 512-block)
            sb, s0 = c0 // SBK, c0 % SBK
            base = sb * NE * SBK + e * SBK + s0
            return xtt[:, base:base + w]
        wqt = wp.tile([128, NE * CH], BF16, tag="wq")
        wkt = wp.tile([128, NE * CH], BF16, tag="wk")
        wvt = wp.tile([128, NE * CH], BF16, tag="wv")
        wqs = [wqt[:, e * CH:(e + 1) * CH] for e in range(NE)]
        wks = [wkt[:, e * CH:(e + 1) * CH] for e in range(NE)]
        wvs = [wvt[:, e * CH:(e + 1) * CH] for e in range(NE)]
        HW_ = NE * CH // 2
        nc.scalar.dma_start(wqt[:, 0:HW_], wq[:, 0:HW_])
        nc.scalar.dma_start(wqt[:, HW_:], wq[:, HW_:])
        nc.gpsimd.dma_start(wkt[:, 0:HW_], wk[:, 0:HW_])
        nc.gpsimd.dma_start(wkt[:, HW_:], wk[:, HW_:])
        BLK = NE * SBK
        QB_ = BLK // 4
        for i in range(4):  # sb0 in quarters (first consumers)
            nc.sync.dma_start(xtt[:, i * QB_:(i + 1) * QB_],
                              xt[:, i * QB_:(i + 1) * QB_])
        nc.gpsimd.dma_start(wvt[:], wv[:, :])
        for sb in range(1, NSB):
            nc.sync.dma_start(xtt[:, sb * BLK:(sb + 1) * BLK],
                              xt[:, sb * BLK:(sb + 1) * BLK])
        x8s, wk8s = [], []
        for c in range(4):
            t = w8p.tile([128, 2 * CH], FP8, tag="wk8")
            nc.gpsimd.dma_start(t[:], wk8[c * 128:(c + 1) * 128, :])
            wk8s.append(t)
        for c in range(4):
            t = x8p.tile([128, 2 * S], FP8, tag="x8")
            nc.sync.dma_start(t[:], xt8[c * 128:(c + 1) * 128, :])
            x8s.append(t)
        wos = []
        for p in range(2):
            t = wop.tile([128, E], BF16, tag="wo")
            nc.gpsimd.dma_start(t[:], wo[p * 128:(p + 1) * 128, :])
            wos.append(t)

        # q/k head-pair tiles [128, S]; vts [128, CH] per key chunk
        qkt = {"q": [qkp.tile([128, S], BF16, tag="qk", name=f"qt{p}")
                     for p in range(2)],
               "k": [qkp.tile([128, S], BF16, tag="qk", name=f"kt{p}")
                     for p in range(2)]}
        vts = [vp.tile([128, CH], BF16, tag="v", name=f"v{i}")
               for i in range(NKC)]
        ons = [onp.tile([128, S], BF16, tag="on", name=f"on{p}")
               for p in range(2)]

        # ---- dense-matmul group emitters (PE filler work) ----
        def qk_group(name, p, sb, pool=None, tag="pp"):
            ps = (pool or ppp).tile([128, SBK], F32, tag=tag,
                                    name=f"ps_{name}{p}{sb}")
            if name == "k" and sb > 0:
                # fp8 DoubleRow: K=256 contraction per MM.  K-storage stays
                # bf16 and k-block 0 stays fully bf16, so few-key rows keep
                # full precision and the l2 noise stays ~1.3e-2.
                for c in range(4):
                    nc.tensor.matmul(
                        ps[:],
                        lhsT=wk8s[c][:].rearrange(
                            "a (i m) -> a i m",
                            i=2)[:, :, p * 128:(p + 1) * 128],
                        rhs=x8s[c][:].rearrange(
                            "a (i s) -> a i s",
                            i=2)[:, :, sb * SBK:(sb + 1) * SBK],
                        start=(c == 0), stop=(c == 3),
                        perf_mode=mybir.MatmulPerfMode.DoubleRow)
            else:
                wts = wqs if name == "q" else wks
                for e in range(NE):
                    nc.tensor.matmul(
                        ps[:], lhsT=wts[e][:, p * 128:(p + 1) * 128],
                        rhs=xsl(e, sb * SBK, SBK),
                        start=(e == 0), stop=(e == NE - 1))
            nc.vector.tensor_copy(qkt[name][p][:, sb * SBK:(sb + 1) * SBK],
                                  ps[:])

        def v_group(sc, pool=None):
            ps = (pool or ppp).tile([128, CH], F32, tag="pp", name=f"ps_v{sc}")
            for e in range(NE):
                nc.tensor.matmul(ps[:], lhsT=xsl(e, sc * 128, 128),
                                 rhs=wvs[e], start=(e == 0),
                                 stop=(e == NE - 1))
            nc.vector.tensor_copy(vts[sc][:], ps[:])

        def wo_group(sc, eb, pool=None, tag="pp", tail=False):
            ps = (pool or ppp).tile([128, SBK], F32, tag=tag,
                                    name=f"ps_o{sc}{eb}")
            for p in range(2):
                nc.tensor.matmul(ps[:],
                                 lhsT=ons[p][:, sc * 128:(sc + 1) * 128],
                                 rhs=wos[p][:, eb * SBK:(eb + 1) * SBK],
                                 start=(p == 0), stop=(p == 1))
            oo = oop.tile([128, SBK], F32, tag="oo", name=f"oo{sc}{eb}")
            if tail and eb % 2 == 0:
                nc.scalar.copy(oo[:], ps[:])
            else:
                nc.vector.tensor_copy(oo[:], ps[:])
            (nc.gpsimd if (tail and eb % 2 == 0) else nc.sync).dma_start(
                pout[sc * 128:(sc + 1) * 128, eb * SBK:(eb + 1) * SBK],
                oo[:])

        from collections import deque
        fillers = deque()  # entries: (tag, est_pe_cost_ns, fn)
        QK_COST, V_COST, WO_COST = 1750, 900, 500

        def pop_fillers(budget_ns):
            while fillers and budget_ns > 0:
                tag, cost, fn = fillers.popleft()
                fn()
                budget_ns -= cost

        def ensure(tag):
            # force-run a queued filler the upcoming consumer depends on
            for i, (t, cost, fn) in enumerate(fillers):
                if t == tag:
                    del fillers[i]
                    fn()
                    return

        # dep-free warm-up: keeps PE at 2.4 GHz while input DMAs land
        warm = denp.tile([128, 512], F32, tag="den", name="warm")
        for _ in range(45):
            nc.tensor.matmul(warm[:, 0:128], lhsT=wrm[:], rhs=wrm[:],
                             start=True, stop=True)
        # prologue: only what (qb0, kc0, pair0) needs; pair1 is ensured later
        qk_group("q", 0, 0, pool=avp, tag="av")
        qk_group("k", 0, 0, pool=avp, tag="av")
        for nm in ("q", "k"):
            fillers.append(
                (("qk", nm, 1, 0), QK_COST,
                 lambda nm=nm: qk_group(nm, 1, 0, pool=avp, tag="av")))
        for sc in range(4):
            fillers.append((("v", sc), V_COST, lambda sc=sc: v_group(sc)))
        for sb in range(1, NSB):
            for nm in ("q", "k"):
                for p in range(2):
                    fillers.append(
                        (("qk", nm, p, sb), QK_COST,
                         lambda nm=nm, p=p, sb=sb: qk_group(nm, p, sb)))

        SCALE = float(D) ** -0.5
        pending_norm = [None]  # deferred normalize emitter for previous qb

        # ---- attention ----
        for qb in range(NSB):
            nk = 4 * (qb + 1)
            if qb > 0:  # guarantee this qb's q/k blocks exist before scores
                for nm in ("q", "k"):
                    for p in range(2):
                        ensure(("qk", nm, p, qb))
            av = [avp.tile([128, SBK], F32, tag="av", name=f"av{qb}{p}")
                  for p in range(2)]
            den = denp.tile([128, 512], F32, tag="den", name=f"den{qb}")
            pend = None  # previous chunk payload: (kc, j0, [pt0, pt1])

            def flush(payload, last=False, av=av, den=den):
                kc, j0, pts = payload
                for p in range(2):
                    for hh in range(2):
                        nc.tensor.matmul(
                            av[p][hh * 64:hh * 64 + 64, j0:SBK],
                            lhsT=vts[kc][:, p * 128 + hh * 64:
                                         p * 128 + hh * 64 + 64],
                            rhs=pts[p][:, hh * SBK + j0:(hh + 1) * SBK],
                            start=False,
                            stop=(last and hh == 1),
                            skip_group_check=True)
                for h in range(HPC):
                    p, hh = h // 2, h % 2
                    nc.tensor.matmul(
                        den[32 * h:32 * h + 1, j0:SBK],
                        lhsT=ones_bf[:, 0:1],
                        rhs=pts[p][:, hh * SBK + j0:(hh + 1) * SBK],
                        start=False,
                        stop=(last and h == HPC - 1),
                        skip_group_check=True,
                        tile_position=(0, 32 * h))

            for kc in range(nk):
                k0 = kc * 128
                j0 = max(0, k0 - qb * SBK)
                pts = []
                for p in range(2):
                    if qb == 0 and kc == 0 and p == 1:
                        ensure(("qk", "q", 1, 0))
                        ensure(("qk", "k", 1, 0))
                    ss = ssp.tile([128, 2 * SBK], F32, tag="ss",
                                  name=f"ss{qb}{kc}{p}")
                    qt, kt = qkt["q"][p], qkt["k"][p]
                    for hh in range(2):
                        nc.tensor.matmul(
                            ss[:, hh * SBK + j0:(hh + 1) * SBK],
                            lhsT=kt[hh * 64:hh * 64 + 64, k0:k0 + 128],
                            rhs=qt[hh * 64:hh * 64 + 64,
                                   qb * SBK + j0:(qb + 1) * SBK],
                            start=True, stop=True)
                    pt = ptp.tile([128, 2 * SBK], BF16, tag="pt",
                                  name=f"pt{qb}{kc}{p}")
                    ss3 = ss[:].rearrange("a (c q) -> a c q", c=2)[:, :, j0:SBK]
                    pt3 = pt[:].rearrange("a (c q) -> a c q", c=2)[:, :, j0:SBK]
                    nc.scalar.activation(pt3, ss3, AF.Exp, scale=SCALE)
                    if k0 >= qb * SBK:  # diag chunk: mask 128-wide band
                        band_pt = pt[:].rearrange(
                            "a (c q) -> a c q", c=2)[:, :, j0:j0 + 128]
                        tri_bc = tri_sb[:].unsqueeze(1).broadcast_to(
                            [128, 2, 128])
                        nc.vector.tensor_mul(band_pt, band_pt, tri_bc)
                    if DEBUG and qb == 0 and kc == 0 and p == 0:
                        nc.sync.dma_start(dbg["dpt"][:, :], pt[:])
                    pts.append(pt)
                if kc == 0:
                    if pending_norm[0] is not None:
                        pending_norm[0]()
                        pending_norm[0] = None
                    nc.vector.memset(av[0][:], 0.0)
                    nc.vector.memset(av[1][:], 0.0)
                    nc.vector.memset(den[:], 0.0)
                if pend is not None:
                    ensure(("v", pend[0]))
                    flush(pend)
                pend = (kc, j0, pts)
                pop_fillers(1900 if qb == 0 else 1150)
            ensure(("v", pend[0]))
            flush(pend, last=True)
            if DEBUG and qb == 0:
                dav_sb = oop.tile([128, SBK], F32, tag="oo", name="dav_sb")
                nc.vector.tensor_copy(dav_sb[:], av[0][:])
                nc.sync.dma_start(dbg["dav"][:, :], dav_sb[:])
                dden_sb = oop.tile([128, SBK], F32, tag="oo", name="dden_sb")
                nc.vector.tensor_copy(dden_sb[:], den[:])
                nc.sync.dma_start(dbg["dden"][:, :], dden_sb[:])
            pop_fillers(700)  # cover the reciprocal latency

            def make_norm(qb=qb, av=av, den=den, last=(qb == NSB - 1)):
                def norm():
                    # recip -> PE bcast matmuls (den bank re-used) -> mul
                    rec = recp.tile([128, 512], F32, tag="rec",
                                    name=f"rec{qb}")
                    nc.vector.reciprocal_approx_fast(rec[:], den[:])
                    for p in range(2):
                        for hh in range(2):
                            h = 2 * p + hh
                            nc.tensor.matmul(
                                den[hh * 64:hh * 64 + 64, :],
                                lhsT=ones_f32[32 * h:32 * h + 1, 0:64],
                                rhs=rec[32 * h:32 * h + 1, :],
                                start=True, stop=True,
                                skip_group_check=True,
                                tile_position=(32 * h, 64 * hh))
                        bc = bcp.tile([128, 512], F32, tag="bc",
                                      name=f"bc{qb}{p}")
                        nc.vector.tensor_copy(bc[:], den[:])
                        if DEBUG and qb == 0 and p == 0:
                            nc.sync.dma_start(dbg["dbc"][:, :], bc[:])
                        nc.vector.tensor_mul(
                            ons[p][:, qb * SBK:(qb + 1) * SBK],
                            av[p][:], bc[:])
                    # wo fillers for this qb are dep-safe only from here on
                    for sc in range(4 * qb, 4 * (qb + 1)):
                        for eb in range(2):
                            fillers.append(
                                (("wo", sc, eb), WO_COST,
                                 lambda sc=sc, eb=eb: wo_group(sc, eb)))
                return norm

            pending_norm[0] = make_norm()
            if qb + 1 < NSB:
                for sc in reversed(range(4 * (qb + 1), 4 * (qb + 2))):
                    fillers.appendleft(
                        (("v", sc), V_COST, lambda sc=sc: v_group(sc)))

        tail_pools = [(ppp, "pp"), (ssp, "ss"), (avp, "av"), (denp, "den"),
                      (ssp, "ss"), (avp, "av")]
        tail_i = [0]

        def tail_wo(sc, eb):
            pool, tg = tail_pools[tail_i[0] % len(tail_pools)]
            tail_i[0] += 1
            wo_group(sc, eb, pool=pool, tag=tg, tail=True)

        pending_norm[0]()
        if DEBUG:
            nc.gpsimd.dma_start(dbg["dqt0"][:, :], qkt["q"][0][:])
            nc.gpsimd.dma_start(dbg["dkt0"][:, :], qkt["k"][0][:])
            nc.sync.dma_start(dbg["dvt0"][:, :], vts[0][:])
            nc.sync.dma_start(dbg["don0"][:, :], ons[0][:])
        while fillers:
            tag, cost, fn = fillers.popleft()
            if tag[0] == "wo":
                tail_wo(tag[1], tag[2])
            else:
                fn()

    nc.compile()
    _BUILT["nc"] = nc
    return nc


def _install_ntff_shim():
    """Provide antenv.axon_hooks (missing in this image) so trace=True works."""
    import types
    try:
        from antenv.axon_hooks import get_axon_ntff_profile_hook  # noqa: F401
        return
    except ImportError:
        pass
    import antenv
    from trn_agent_boot.trn_boot import _ntff_profile_via_ctypes
    hook = _ntff_profile_via_ctypes("/opt/axon/libaxon_pjrt.so")
    mod = types.ModuleType("antenv.axon_hooks")
    mod._hook = hook
    mod.get_axon_ntff_profile_hook = lambda: mod._hook
    mod.set_axon_ntff_profile_hook = lambda h: setattr(mod, "_hook", h)
    sys.modules["antenv.axon_hooks"] = mod
    antenv.axon_hooks = mod


def kernel(x, Wq, Wk, Wv, Wo, bo, _trace=False):
    from concourse.bass_utils import run_bass_kernel_spmd

    nc = _build()

    x = np.asarray(x, dtype=np.float32)
    Wq = np.asarray(Wq, dtype=np.float32)
    Wk = np.asarray(Wk, dtype=np.float32)
    Wv = np.asarray(Wv, dtype=np.float32)
    Wo = np.asarray(Wo, dtype=np.float32)
    bo = np.asarray(bo, dtype=np.float32)

    import ml_dtypes
    bf = ml_dtypes.bfloat16
    tri = np.triu(np.ones((128, 128), dtype=np.float32)).astype(bf)

    def pack8(a):
        # [1024, C] -> [128, 8*C]: row 128c+p -> col c*C+j
        C = a.shape[1]
        return np.ascontiguousarray(
            a.reshape(8, 128, C).transpose(1, 0, 2).reshape(128, 8 * C))

    def packx(a):  # x^T [E,S] -> [128, (sb, e, s)] sb-major
        return np.ascontiguousarray(
            a.reshape(8, 128, 4, 512).transpose(1, 2, 0, 3).reshape(
                128, 8 * 2048))

    f8 = ml_dtypes.float8_e4m3fn

    def pack_dr(a):
        # rows r=256c+128i+p -> [c*128+p, i*C+col] (DoubleRow operand layout)
        K, C = a.shape[0] // 256, a.shape[1]
        return np.ascontiguousarray(
            a.reshape(K, 2, 128, C).transpose(0, 2, 1, 3).reshape(
                K * 128, 2 * C))

    xt_b = [packx(np.ascontiguousarray(x[b].T)).astype(bf) for b in range(B)]
    xt8_b = [pack_dr(np.ascontiguousarray(x[b].T)).astype(f8)
             for b in range(B)]
    in_maps = []
    for c in range(N_CORES):
        b, hg = c // HPC, c % HPC
        sl = slice(hg * CH, (hg + 1) * CH)
        in_maps.append({
            "xt": xt_b[b],
            "wq": pack8(np.ascontiguousarray(Wq[:, sl])).astype(bf),
            "wk": pack8(np.ascontiguousarray(Wk[:, sl])).astype(bf),
            "wv": pack8(np.ascontiguousarray(Wv[:, sl])).astype(bf),
            "wk8": pack_dr(np.ascontiguousarray(Wk[:, sl])).astype(f8),
            "xt8": xt8_b[b],
            "wo": np.ascontiguousarray(Wo[sl, :]).astype(bf),
            "tri": tri,
        })

    kwargs = {}
    if _trace:
        _install_ntff_shim()
        kwargs = dict(trace=True, trace_cores=[0])
    res = run_bass_kernel_spmd(nc, in_maps, core_ids=list(range(N_CORES)),
                               **kwargs)

    out = np.zeros((B, S, E), dtype=np.float32)
    for c in range(N_CORES):
        out[c // HPC] += res.results[c]["pout"]
    out += bo
    if _trace:
        return out, res
    return out
